# revision 6
# baseline (speedup 1.0000x reference)
"""CQAttention (QANet context-query attention) on 8 Trainium2 NeuronCores.

Full inputs in, full output out. Data-parallel over batch B=32 -> 4 batches
per core. See _build_program() for the per-core Bass/Tile program.

Math notes (vs the jax reference):
  - `bias` and the cross-terms sub0/sub1 that are constant along a softmax
    axis drop out of that softmax; sub1 enters S1's logits as a per-q bias,
    sub0 enters S2's logits via folding w4C into the rhs of the S2 matmul.
  - S1 = softmax_q(sub2 + sub1 + Qmaskbias): computed transposed [q, c];
    row-sum rs[c] over q via an all-ones [q,128] lhsT matmul (result arrives
    pre-broadcast over 128 partitions); 1/rs applied to the final A^T/Bt^T.
  - S2 = softmax_c(sub2 + sub0 + Cmaskbias): computed [c, q]; the c-mask is
    applied multiplicatively (on transposed C rows and on the column-sum
    matmul's lhsT), so exp needs no per-chunk bias.
  - A^T = Qt^T-weighted sums, Bt^T = S1t-weighted V, V = S2^T @ Ct, all via
    PE matmuls with the contraction dim on partitions.
"""

import os
import sys

for _p in ("/opt/trn_rl_repo", "/root/.axon_site/_ro/trn_rl_repo"):
    if os.path.isdir(_p) and _p not in sys.path:
        sys.path.insert(0, _p)

import numpy as np

N_CORES = 8
B_FULL = 32
BPC = B_FULL // N_CORES  # batches per core
D = 128
LC = 2048
LQ = 256
NEG_BIG = -30000.0

_CACHE = {}


def _build_program():
    import concourse.mybir as mybir
    import concourse.tile as tile
    from concourse import bacc
    from concourse.masks import make_identity

    f32 = mybir.dt.float32
    f32r = mybir.dt.float32r
    AF = mybir.ActivationFunctionType
    OP = mybir.AluOpType

    nc = bacc.Bacc("TRN2", target_bir_lowering=False, debug=False)

    Cd = nc.dram_tensor("C", [BPC, D, LC], f32, kind="ExternalInput")
    Qd = nc.dram_tensor("Q", [BPC, D, LQ], f32r, kind="ExternalInput")
    nQd = nc.dram_tensor("negQm", [BPC, D, 2], f32, kind="ExternalInput")
    Cmd = nc.dram_tensor("Cmf", [BPC, D, 16], f32r, kind="ExternalInput")
    wmlud = nc.dram_tensor("wmlu", [D, 1], f32, kind="ExternalInput")
    wcd = nc.dram_tensor("wc", [D, 1], f32, kind="ExternalInput")
    wqd = nc.dram_tensor("wq", [D, 1], f32, kind="ExternalInput")
    outd = nc.dram_tensor("out", [BPC, 4 * D, LC], f32, kind="ExternalOutput")

    def v32(ap):
        return ap.bitcast(f32)

    with tile.TileContext(nc) as tc:
        with (
            tc.tile_pool(name="const", bufs=1) as constp,
            tc.tile_pool(name="big", bufs=2) as sb,
            tc.tile_pool(name="small", bufs=2) as sbs,
            tc.tile_pool(name="psbig", bufs=2, space="PSUM") as psbig,
            tc.tile_pool(name="psct", bufs=2, space="PSUM") as psct,
            tc.tile_pool(name="pssm", bufs=2, space="PSUM") as pssm,
            tc.tile_pool(name="dram", bufs=2, space="DRAM") as dramp,
        ):
            ident = constp.tile([128, 128], f32)
            make_identity(nc, ident[:])
            ones32 = constp.tile([128, 128], f32)
            nc.vector.memset(ones32[:], 1.0)
            onesm = constp.tile([128, 128], f32r)
            nc.vector.tensor_copy(onesm[:], ones32[:])
            wmlu = constp.tile([D, 1], f32)
            nc.sync.dma_start(out=wmlu[:], in_=wmlud.ap())
            wc = constp.tile([D, 1], f32)
            nc.sync.dma_start(out=wc[:], in_=wcd.ap())
            wq = constp.tile([D, 1], f32)
            nc.sync.dma_start(out=wq[:], in_=wqd.ap())

            for b in range(BPC):
                # ---------------- loads ----------------
                Cb = sb.tile([128, LC], f32, tag="Cb")
                nc.sync.dma_start(out=Cb[:], in_=Cd.ap()[b, :, :])
                Qb = sbs.tile([128, LQ], f32r, tag="Qb")
                nc.sync.dma_start(out=Qb[:], in_=Qd.ap()[b, :, :])
                nQm = sbs.tile([128, 2], f32, tag="nQm")
                nc.sync.dma_start(out=nQm[:], in_=nQd.ap()[b, :, :])
                Cmc = sbs.tile([128, 16], f32r, tag="Cmc")
                nc.sync.dma_start(out=Cmc[:], in_=Cmd.ap()[b, :, :])

                # rounded copy of C for f32r matmul consumption
                Cb_r = sb.tile([128, LC], f32r, tag="Cb_r")
                nc.vector.tensor_copy(Cb_r[:], Cb[:])

                # ---------------- small prep ----------------
                QbW = sbs.tile([128, LQ], f32r, tag="QbW")
                nc.vector.tensor_scalar_mul(out=QbW[:], in0=v32(Qb[:]), scalar1=wmlu[:])
                Qw = sbs.tile([128, LQ], f32r, tag="Qw")
                nc.vector.tensor_scalar(
                    out=Qw[:], in0=v32(Qb[:]), scalar1=wmlu[:], scalar2=wc[:],
                    op0=OP.mult, op1=OP.add,
                )

                # sub1[q] = sum_d Q[d,q] * w4Q[d]  -> [q, 1] per q-chunk
                ps_sub1 = pssm.tile([128, 2], f32, tag="sm")
                for qj in range(2):
                    nc.tensor.matmul(
                        ps_sub1[:, qj : qj + 1],
                        lhsT=v32(Qb[:, 128 * qj : 128 * (qj + 1)]),
                        rhs=wq[:],
                        start=True, stop=True,
                    )
                biasQ = sbs.tile([128, 2], f32, tag="biasQ")
                nc.vector.tensor_add(out=biasQ[:], in0=nQm[:], in1=ps_sub1[:])

                # Qt [q, d] (2 chunks side by side)
                ps_qt = pssm.tile([128, 256], f32, tag="sm")
                for qj in range(2):
                    nc.tensor.transpose(
                        ps_qt[:, 128 * qj : 128 * (qj + 1)],
                        in_=v32(Qb[:, 128 * qj : 128 * (qj + 1)]),
                        identity=ident[:],
                    )
                QtS = sbs.tile([128, 256], f32r, tag="QtS")
                nc.vector.tensor_copy(QtS[:], ps_qt[:])

                # ---------------- CT (transposed, c-masked C) ----------------
                CTm = []
                for g in range(4):
                    ps_ct = psct.tile([128, 512], f32, tag="ct")
                    for k in range(4):
                        j = 4 * g + k
                        nc.tensor.transpose(
                            ps_ct[:, 128 * k : 128 * (k + 1)],
                            in_=Cb[:, 128 * j : 128 * (j + 1)],
                            identity=ident[:],
                        )
                    ctm = sb.tile([128, 512], f32r, tag=f"CTm{g}")
                    if g % 2 == 0:
                        nc.scalar.copy(out=ctm[:], in_=ps_ct[:])
                    else:
                        nc.vector.tensor_copy(ctm[:], ps_ct[:])
                    for k in range(4):
                        j = 4 * g + k
                        nc.vector.tensor_scalar_mul(
                            out=ctm[:, 128 * k : 128 * (k + 1)],
                            in0=v32(ctm[:, 128 * k : 128 * (k + 1)]),
                            scalar1=v32(Cmc[:, j : j + 1]),
                        )
                    CTm.append(ctm)

                # ---------------- S1 side: N1t [q, c] ----------------
                N1t = []
                for qj in range(2):
                    n1 = sb.tile([128, LC], f32r, tag=f"N1t{qj}")
                    for h in range(2):
                        ps = psbig.tile([128, 1024], f32, tag="bigmm")
                        for n5 in range(2):
                            c0 = 1024 * h + 512 * n5
                            nc.tensor.matmul(
                                ps[:, 512 * n5 : 512 * (n5 + 1)],
                                lhsT=QbW[:, 128 * qj : 128 * (qj + 1)],
                                rhs=Cb_r[:, c0 : c0 + 512],
                                start=True, stop=True,
                            )
                        nc.scalar.activation(
                            out=n1[:, 1024 * h : 1024 * (h + 1)],
                            in_=ps[:],
                            func=AF.Exp,
                            bias=biasQ[:, qj : qj + 1],
                            scale=1.0,
                        )
                    N1t.append(n1)

                # rs[c] broadcast over partitions, then 1/rs
                RBr = sb.tile([128, LC], f32, tag="RBr")
                for h in range(2):
                    ps = psbig.tile([128, 1024], f32, tag="bigmm")
                    for n5 in range(2):
                        c0 = 1024 * h + 512 * n5
                        for qj in range(2):
                            nc.tensor.matmul(
                                ps[:, 512 * n5 : 512 * (n5 + 1)],
                                lhsT=onesm[:],
                                rhs=N1t[qj][:, c0 : c0 + 512],
                                start=(qj == 0), stop=(qj == 1),
                            )
                    nc.vector.reciprocal_approx_fast(
                        out=RBr[:, 1024 * h : 1024 * (h + 1)], in_=ps[:]
                    )

                # ---------------- S2 side: N2 [c, q] ----------------
                N2 = []
                for s in range(2):
                    n2 = sb.tile([128, 8, 256], f32r, tag=f"N2{s}")
                    for h in range(2):
                        ps = psbig.tile([128, 1024], f32, tag="bigmm")
                        for k in range(4):
                            j = 8 * s + 4 * h + k
                            nc.tensor.matmul(
                                ps[:, 256 * k : 256 * (k + 1)],
                                lhsT=Cb_r[:, 128 * j : 128 * (j + 1)],
                                rhs=Qw[:],
                                start=True, stop=True,
                            )
                        nc.scalar.activation(
                            out=n2[:, 4 * h : 4 * (h + 1), :],
                            in_=ps[:],
                            func=AF.Exp,
                        )
                    N2.append(n2)

                # cs[q] = sum_c Cm[c] * N2[c, q]  -> [1, 256] psum
                ps_cs = pssm.tile([1, 256], f32, tag="sm")
                for j in range(16):
                    s, jj = divmod(j, 8)
                    nc.tensor.matmul(
                        ps_cs[:],
                        lhsT=Cmc[:, j : j + 1],
                        rhs=N2[s][:, jj, :],
                        start=(j == 0), stop=(j == 15),
                    )
                cs_row = sbs.tile([1, 256], f32, tag="cs_row")
                nc.vector.tensor_copy(cs_row[:], ps_cs[:])
                # reshape [1, 256] -> [128, 2] via a DRAM bounce (strided DRAM
                # read; SBUF partition dim cannot be re-strided)
                dcs = dramp.tile([1, 256], f32, tag="dcs")
                nc.sync.dma_start(out=dcs[:], in_=cs_row[:])
                cs_col = sbs.tile([128, 2], f32, tag="cs_col")
                nc.sync.dma_start(
                    out=cs_col[:],
                    in_=dcs[:].rearrange("a (j p) -> p (a j)", p=128),
                )
                rcs = sbs.tile([128, 2], f32, tag="rcs")
                nc.vector.reciprocal(out=rcs[:], in_=cs_col[:])

                # ---------------- V = S2^T @ Ct  [q, d] ----------------
                ps_vt = pssm.tile([128, 256], f32, tag="sm")
                for j in range(16):
                    s, jj = divmod(j, 8)
                    g, k = divmod(j, 4)
                    nc.tensor.matmul(
                        ps_vt[:],
                        lhsT=CTm[g][:, 128 * k : 128 * (k + 1)],
                        rhs=N2[s][:, jj, :],
                        start=(j == 0), stop=(j == 15),
                    )
                VtS = sbs.tile([128, 256], f32, tag="VtS")
                nc.vector.tensor_copy(VtS[:], ps_vt[:])
                ps_v = pssm.tile([128, 256], f32, tag="sm")
                for qj in range(2):
                    nc.tensor.transpose(
                        ps_v[:, 128 * qj : 128 * (qj + 1)],
                        in_=VtS[:, 128 * qj : 128 * (qj + 1)],
                        identity=ident[:],
                    )
                Vs = sbs.tile([128, 256], f32r, tag="Vs")
                for qj in range(2):
                    nc.vector.tensor_scalar_mul(
                        out=Vs[:, 128 * qj : 128 * (qj + 1)],
                        in0=ps_v[:, 128 * qj : 128 * (qj + 1)],
                        scalar1=rcs[:, qj : qj + 1],
                    )

                # ---------------- outputs ----------------
                # out row-block 0: C itself
                nc.sync.dma_start(out=outd.ap()[b, 0:128, :], in_=Cb[:])

                # A^T and Bt^T raw matmuls + normalization + C products
                o2 = sb.tile([128, LC], f32, tag="o2")
                o4a = sb.tile([128, LC], f32, tag="o4a")
                for h in range(2):
                    ps_at = psbig.tile([128, 1024], f32, tag="bigmm")
                    for n5 in range(2):
                        c0 = 1024 * h + 512 * n5
                        for qj in range(2):
                            nc.tensor.matmul(
                                ps_at[:, 512 * n5 : 512 * (n5 + 1)],
                                lhsT=QtS[:, 128 * qj : 128 * (qj + 1)],
                                rhs=N1t[qj][:, c0 : c0 + 512],
                                start=(qj == 0), stop=(qj == 1),
                            )
                    nc.vector.tensor_mul(
                        out=o2[:, 1024 * h : 1024 * (h + 1)],
                        in0=ps_at[:],
                        in1=RBr[:, 1024 * h : 1024 * (h + 1)],
                    )
                for h in range(2):
                    ps_bt = psbig.tile([128, 1024], f32, tag="bigmm")
                    for n5 in range(2):
                        c0 = 1024 * h + 512 * n5
                        for qj in range(2):
                            nc.tensor.matmul(
                                ps_bt[:, 512 * n5 : 512 * (n5 + 1)],
                                lhsT=Vs[:, 128 * qj : 128 * (qj + 1)],
                                rhs=N1t[qj][:, c0 : c0 + 512],
                                start=(qj == 0), stop=(qj == 1),
                            )
                    nc.vector.tensor_mul(
                        out=o4a[:, 1024 * h : 1024 * (h + 1)],
                        in0=ps_bt[:],
                        in1=RBr[:, 1024 * h : 1024 * (h + 1)],
                    )
                nc.sync.dma_start(out=outd.ap()[b, 128:256, :], in_=o2[:])

                o3 = sb.tile([128, LC], f32, tag="o3", bufs=1)
                nc.vector.tensor_mul(out=o3[:], in0=o2[:], in1=Cb[:])
                nc.sync.dma_start(out=outd.ap()[b, 256:384, :], in_=o3[:])

                o4 = sb.tile([128, LC], f32, tag="o4", bufs=1)
                nc.vector.tensor_mul(out=o4[:], in0=o4a[:], in1=Cb[:])
                nc.sync.dma_start(out=outd.ap()[b, 384:512, :], in_=o4[:])

    nc.compile()
    return nc


def _get_program():
    if "nc" not in _CACHE:
        _CACHE["nc"] = _build_program()
    return _CACHE["nc"]


def _shard_inputs(C, Q, Cmask, Qmask, w4C, w4Q, w4mlu):
    C = np.ascontiguousarray(C, dtype=np.float32)
    Q = np.ascontiguousarray(Q, dtype=np.float32)
    negQm = (NEG_BIG * (1.0 - Qmask.astype(np.float32))).astype(np.float32)
    # [B, LQ] -> [B, 2, 128] -> [B, 128, 2]
    negQm = np.ascontiguousarray(negQm.reshape(B_FULL, 2, 128).transpose(0, 2, 1))
    Cmf = Cmask.astype(np.float32).reshape(B_FULL, 16, 128).transpose(0, 2, 1)
    Cmf = np.ascontiguousarray(Cmf)
    wmlu = np.ascontiguousarray(np.asarray(w4mlu, dtype=np.float32).reshape(D, 1))
    wc = np.ascontiguousarray(np.asarray(w4C, dtype=np.float32).reshape(D, 1))
    wq = np.ascontiguousarray(np.asarray(w4Q, dtype=np.float32).reshape(D, 1))
    in_maps = []
    for i in range(N_CORES):
        sl = slice(BPC * i, BPC * (i + 1))
        in_maps.append(
            {
                "C": C[sl],
                "Q": Q[sl],
                "negQm": negQm[sl],
                "Cmf": Cmf[sl],
                "wmlu": wmlu,
                "wc": wc,
                "wq": wq,
            }
        )
    return in_maps


def kernel(C, Q, Cmask, Qmask, w4C, w4Q, w4mlu, bias):
    # bias is a scalar added to every logit; it cancels in both softmaxes and
    # never reaches the output, so it is accepted and ignored.
    from concourse.bass_utils import run_bass_kernel_spmd

    nc = _get_program()
    in_maps = _shard_inputs(C, Q, Cmask, Qmask, w4C, w4Q, w4mlu)
    res = run_bass_kernel_spmd(nc, in_maps, list(range(N_CORES)))
    out = np.concatenate([res.results[i]["out"] for i in range(N_CORES)], axis=0)
    return out.astype(np.float32)


# revision 7
# speedup vs baseline: 5.8586x; 5.8586x over previous
"""CQAttention (QANet context-query attention) on 8 Trainium2 NeuronCores.

Full inputs in, full output out. Data-parallel over batch B=32 -> 4 batches
per core. See _build_program() for the per-core Bass/Tile program.

Math notes (vs the jax reference):
  - `bias` and the cross-terms sub0/sub1 that are constant along a softmax
    axis drop out of that softmax; sub1 enters S1's logits as a per-q bias,
    sub0 enters S2's logits via folding w4C into the rhs of the S2 matmul.
  - S1 = softmax_q(sub2 + sub1 + Qmaskbias): computed transposed [q, c];
    row-sum rs[c] over q via an all-ones [q,128] lhsT matmul (result arrives
    pre-broadcast over 128 partitions); 1/rs applied to the final A^T/Bt^T.
  - S2 = softmax_c(sub2 + sub0 + Cmaskbias): computed [c, q]; the c-mask is
    applied multiplicatively (on transposed C rows and on the column-sum
    matmul's lhsT), so exp needs no per-chunk bias.
  - A^T = Qt^T-weighted sums, Bt^T = S1t-weighted V, V = S2^T @ Ct, all via
    PE matmuls with the contraction dim on partitions.
"""

import os
import sys

for _p in ("/opt/trn_rl_repo", "/root/.axon_site/_ro/trn_rl_repo"):
    if os.path.isdir(_p) and _p not in sys.path:
        sys.path.insert(0, _p)

import numpy as np

N_CORES = 8
B_FULL = 32
BPC = B_FULL // N_CORES  # batches per core
D = 128
LC = 2048
LQ = 256
NEG_BIG = -30000.0

_CACHE = {}


def _build_program(repeat=1):
    import concourse.mybir as mybir
    import concourse.tile as tile
    from concourse import bacc
    from concourse.masks import make_identity

    f32 = mybir.dt.float32
    f32r = mybir.dt.float32r
    AF = mybir.ActivationFunctionType
    OP = mybir.AluOpType

    nc = bacc.Bacc("TRN2", target_bir_lowering=False, debug=False)

    Cd = nc.dram_tensor("C", [BPC, D, LC], f32, kind="ExternalInput")
    Qd = nc.dram_tensor("Q", [BPC, D, LQ], f32r, kind="ExternalInput")
    nQd = nc.dram_tensor("negQm", [BPC, D, 2], f32, kind="ExternalInput")
    Cmd = nc.dram_tensor("Cmf", [BPC, D, 16], f32r, kind="ExternalInput")
    wmlud = nc.dram_tensor("wmlu", [D, 1], f32, kind="ExternalInput")
    wcd = nc.dram_tensor("wc", [D, 1], f32, kind="ExternalInput")
    wqd = nc.dram_tensor("wq", [D, 1], f32, kind="ExternalInput")
    outd = nc.dram_tensor("out", [BPC, 4 * D, LC], f32, kind="ExternalOutput")

    def v32(ap):
        return ap.bitcast(f32)

    with tile.TileContext(nc) as tc:
        with (
            tc.tile_pool(name="const", bufs=1) as constp,
            tc.tile_pool(name="big", bufs=2) as sb,
            tc.tile_pool(name="small", bufs=2) as sbs,
            tc.tile_pool(name="psbig", bufs=2, space="PSUM") as psbig,
            tc.tile_pool(name="psct", bufs=2, space="PSUM") as psct,
            tc.tile_pool(name="pssm", bufs=2, space="PSUM") as pssm,
            tc.tile_pool(name="dram", bufs=2, space="DRAM") as dramp,
        ):
            ident = constp.tile([128, 128], f32)
            make_identity(nc, ident[:])
            ones32 = constp.tile([128, 128], f32)
            nc.vector.memset(ones32[:], 1.0)
            onesm = constp.tile([128, 128], f32r)
            nc.vector.tensor_copy(onesm[:], ones32[:])
            wmlu = constp.tile([D, 1], f32)
            nc.sync.dma_start(out=wmlu[:], in_=wmlud.ap())
            wc = constp.tile([D, 1], f32)
            nc.sync.dma_start(out=wc[:], in_=wcd.ap())
            wq = constp.tile([D, 1], f32)
            nc.sync.dma_start(out=wq[:], in_=wqd.ap())

            for b0 in range(repeat * BPC):
                b = b0 % BPC
                # ---------------- loads ----------------
                Cb = sb.tile([128, LC], f32, tag="Cb")
                nc.sync.dma_start(out=Cb[:], in_=Cd.ap()[b, :, :])
                Qb = sbs.tile([128, LQ], f32r, tag="Qb")
                nc.sync.dma_start(out=Qb[:], in_=Qd.ap()[b, :, :])
                nQm = sbs.tile([128, 2], f32, tag="nQm")
                nc.sync.dma_start(out=nQm[:], in_=nQd.ap()[b, :, :])
                Cmc = sbs.tile([128, 16], f32r, tag="Cmc")
                nc.sync.dma_start(out=Cmc[:], in_=Cmd.ap()[b, :, :])

                # rounded copy of C for f32r matmul consumption
                Cb_r = sb.tile([128, LC], f32r, tag="Cb_r")
                nc.vector.tensor_copy(Cb_r[:], Cb[:])

                # ---------------- small prep ----------------
                QbW = sbs.tile([128, LQ], f32r, tag="QbW")
                nc.vector.tensor_scalar_mul(out=QbW[:], in0=v32(Qb[:]), scalar1=wmlu[:])
                Qw = sbs.tile([128, LQ], f32r, tag="Qw")
                nc.vector.tensor_scalar(
                    out=Qw[:], in0=v32(Qb[:]), scalar1=wmlu[:], scalar2=wc[:],
                    op0=OP.mult, op1=OP.add,
                )

                # sub1[q] = sum_d Q[d,q] * w4Q[d]  -> [q, 1] per q-chunk
                ps_sub1 = pssm.tile([128, 2], f32, tag="sm")
                for qj in range(2):
                    nc.tensor.matmul(
                        ps_sub1[:, qj : qj + 1],
                        lhsT=v32(Qb[:, 128 * qj : 128 * (qj + 1)]),
                        rhs=wq[:],
                        start=True, stop=True,
                    )
                biasQ = sbs.tile([128, 2], f32, tag="biasQ")
                nc.vector.tensor_add(out=biasQ[:], in0=nQm[:], in1=ps_sub1[:])

                # Qt [q, d] (2 chunks side by side)
                ps_qt = pssm.tile([128, 256], f32, tag="sm")
                for qj in range(2):
                    nc.tensor.transpose(
                        ps_qt[:, 128 * qj : 128 * (qj + 1)],
                        in_=v32(Qb[:, 128 * qj : 128 * (qj + 1)]),
                        identity=ident[:],
                    )
                QtS = sbs.tile([128, 256], f32r, tag="QtS")
                nc.vector.tensor_copy(QtS[:], ps_qt[:])

                # ---------------- CT (transposed, c-masked C) ----------------
                CTm = []
                for g in range(4):
                    ps_ct = psct.tile([128, 512], f32, tag="ct")
                    for k in range(4):
                        j = 4 * g + k
                        nc.tensor.transpose(
                            ps_ct[:, 128 * k : 128 * (k + 1)],
                            in_=Cb[:, 128 * j : 128 * (j + 1)],
                            identity=ident[:],
                        )
                    ctm = sb.tile([128, 512], f32r, tag=f"CTm{g}")
                    if g % 2 == 0:
                        nc.scalar.copy(out=ctm[:], in_=ps_ct[:])
                    else:
                        nc.vector.tensor_copy(ctm[:], ps_ct[:])
                    for k in range(4):
                        j = 4 * g + k
                        nc.vector.tensor_scalar_mul(
                            out=ctm[:, 128 * k : 128 * (k + 1)],
                            in0=v32(ctm[:, 128 * k : 128 * (k + 1)]),
                            scalar1=v32(Cmc[:, j : j + 1]),
                        )
                    CTm.append(ctm)

                # ---------------- S1 side: N1t [q, c] ----------------
                N1t = []
                for qj in range(2):
                    n1 = sb.tile([128, LC], f32r, tag=f"N1t{qj}")
                    for h in range(2):
                        ps = psbig.tile([128, 1024], f32, tag="bigmm")
                        for n5 in range(2):
                            c0 = 1024 * h + 512 * n5
                            nc.tensor.matmul(
                                ps[:, 512 * n5 : 512 * (n5 + 1)],
                                lhsT=QbW[:, 128 * qj : 128 * (qj + 1)],
                                rhs=Cb_r[:, c0 : c0 + 512],
                                start=True, stop=True,
                            )
                        nc.scalar.activation(
                            out=n1[:, 1024 * h : 1024 * (h + 1)],
                            in_=ps[:],
                            func=AF.Exp,
                            bias=biasQ[:, qj : qj + 1],
                            scale=1.0,
                        )
                    N1t.append(n1)

                # rs[c] broadcast over partitions, then 1/rs
                RBr = sb.tile([128, LC], f32, tag="RBr")
                for h in range(2):
                    ps = psbig.tile([128, 1024], f32, tag="bigmm")
                    for n5 in range(2):
                        c0 = 1024 * h + 512 * n5
                        for qj in range(2):
                            nc.tensor.matmul(
                                ps[:, 512 * n5 : 512 * (n5 + 1)],
                                lhsT=onesm[:],
                                rhs=N1t[qj][:, c0 : c0 + 512],
                                start=(qj == 0), stop=(qj == 1),
                            )
                    nc.vector.reciprocal_approx_fast(
                        out=RBr[:, 1024 * h : 1024 * (h + 1)], in_=ps[:]
                    )

                # ---------------- S2 side: N2 [c, q] ----------------
                N2 = []
                for s in range(2):
                    n2 = sb.tile([128, 8, 256], f32r, tag=f"N2{s}")
                    for h in range(2):
                        ps = psbig.tile([128, 1024], f32, tag="bigmm")
                        for k in range(4):
                            j = 8 * s + 4 * h + k
                            nc.tensor.matmul(
                                ps[:, 256 * k : 256 * (k + 1)],
                                lhsT=Cb_r[:, 128 * j : 128 * (j + 1)],
                                rhs=Qw[:],
                                start=True, stop=True,
                            )
                        nc.scalar.activation(
                            out=n2[:, 4 * h : 4 * (h + 1), :],
                            in_=ps[:],
                            func=AF.Exp,
                        )
                    N2.append(n2)

                # cs[q] = sum_c Cm[c] * N2[c, q]  -> [1, 256] psum
                ps_cs = pssm.tile([1, 256], f32, tag="sm")
                for j in range(16):
                    s, jj = divmod(j, 8)
                    nc.tensor.matmul(
                        ps_cs[:],
                        lhsT=Cmc[:, j : j + 1],
                        rhs=N2[s][:, jj, :],
                        start=(j == 0), stop=(j == 15),
                    )
                cs_row = sbs.tile([1, 256], f32, tag="cs_row")
                nc.vector.tensor_copy(cs_row[:], ps_cs[:])
                # reshape [1, 256] -> [128, 2] via a DRAM bounce (strided DRAM
                # read; SBUF partition dim cannot be re-strided)
                dcs = dramp.tile([1, 256], f32, tag="dcs")
                nc.sync.dma_start(out=dcs[:], in_=cs_row[:])
                cs_col = sbs.tile([128, 2], f32, tag="cs_col")
                nc.sync.dma_start(
                    out=cs_col[:],
                    in_=dcs[:].rearrange("a (j p) -> p (a j)", p=128),
                )
                rcs = sbs.tile([128, 2], f32, tag="rcs")
                nc.vector.reciprocal(out=rcs[:], in_=cs_col[:])

                # ---------------- V = S2^T @ Ct  [q, d] ----------------
                ps_vt = pssm.tile([128, 256], f32, tag="sm")
                for j in range(16):
                    s, jj = divmod(j, 8)
                    g, k = divmod(j, 4)
                    nc.tensor.matmul(
                        ps_vt[:],
                        lhsT=CTm[g][:, 128 * k : 128 * (k + 1)],
                        rhs=N2[s][:, jj, :],
                        start=(j == 0), stop=(j == 15),
                    )
                VtS = sbs.tile([128, 256], f32, tag="VtS")
                nc.vector.tensor_copy(VtS[:], ps_vt[:])
                ps_v = pssm.tile([128, 256], f32, tag="sm")
                for qj in range(2):
                    nc.tensor.transpose(
                        ps_v[:, 128 * qj : 128 * (qj + 1)],
                        in_=VtS[:, 128 * qj : 128 * (qj + 1)],
                        identity=ident[:],
                    )
                Vs = sbs.tile([128, 256], f32r, tag="Vs")
                for qj in range(2):
                    nc.vector.tensor_scalar_mul(
                        out=Vs[:, 128 * qj : 128 * (qj + 1)],
                        in0=ps_v[:, 128 * qj : 128 * (qj + 1)],
                        scalar1=rcs[:, qj : qj + 1],
                    )

                # ---------------- outputs ----------------
                # out row-block 0: C itself
                nc.sync.dma_start(out=outd.ap()[b, 0:128, :], in_=Cb[:])

                # A^T and Bt^T raw matmuls + normalization + C products
                o2 = sb.tile([128, LC], f32, tag="o2")
                o4a = sb.tile([128, LC], f32, tag="o4a")
                for h in range(2):
                    ps_at = psbig.tile([128, 1024], f32, tag="bigmm")
                    for n5 in range(2):
                        c0 = 1024 * h + 512 * n5
                        for qj in range(2):
                            nc.tensor.matmul(
                                ps_at[:, 512 * n5 : 512 * (n5 + 1)],
                                lhsT=QtS[:, 128 * qj : 128 * (qj + 1)],
                                rhs=N1t[qj][:, c0 : c0 + 512],
                                start=(qj == 0), stop=(qj == 1),
                            )
                    nc.vector.tensor_mul(
                        out=o2[:, 1024 * h : 1024 * (h + 1)],
                        in0=ps_at[:],
                        in1=RBr[:, 1024 * h : 1024 * (h + 1)],
                    )
                for h in range(2):
                    ps_bt = psbig.tile([128, 1024], f32, tag="bigmm")
                    for n5 in range(2):
                        c0 = 1024 * h + 512 * n5
                        for qj in range(2):
                            nc.tensor.matmul(
                                ps_bt[:, 512 * n5 : 512 * (n5 + 1)],
                                lhsT=Vs[:, 128 * qj : 128 * (qj + 1)],
                                rhs=N1t[qj][:, c0 : c0 + 512],
                                start=(qj == 0), stop=(qj == 1),
                            )
                    nc.vector.tensor_mul(
                        out=o4a[:, 1024 * h : 1024 * (h + 1)],
                        in0=ps_bt[:],
                        in1=RBr[:, 1024 * h : 1024 * (h + 1)],
                    )
                nc.sync.dma_start(out=outd.ap()[b, 128:256, :], in_=o2[:])

                o3 = sb.tile([128, LC], f32, tag="o3", bufs=1)
                nc.vector.tensor_mul(out=o3[:], in0=o2[:], in1=Cb[:])
                nc.sync.dma_start(out=outd.ap()[b, 256:384, :], in_=o3[:])

                o4 = sb.tile([128, LC], f32, tag="o4", bufs=1)
                nc.vector.tensor_mul(out=o4[:], in0=o4a[:], in1=Cb[:])
                nc.sync.dma_start(out=outd.ap()[b, 384:512, :], in_=o4[:])

    nc.compile()
    return nc


def _get_program(repeat=1):
    key = f"nc{repeat}"
    if key not in _CACHE:
        _CACHE[key] = _build_program(repeat)
    return _CACHE[key]


def _shard_inputs(C, Q, Cmask, Qmask, w4C, w4Q, w4mlu):
    C = np.ascontiguousarray(C, dtype=np.float32)
    Q = np.ascontiguousarray(Q, dtype=np.float32)
    negQm = (NEG_BIG * (1.0 - Qmask.astype(np.float32))).astype(np.float32)
    # [B, LQ] -> [B, 2, 128] -> [B, 128, 2]
    negQm = np.ascontiguousarray(negQm.reshape(B_FULL, 2, 128).transpose(0, 2, 1))
    Cmf = Cmask.astype(np.float32).reshape(B_FULL, 16, 128).transpose(0, 2, 1)
    Cmf = np.ascontiguousarray(Cmf)
    wmlu = np.ascontiguousarray(np.asarray(w4mlu, dtype=np.float32).reshape(D, 1))
    wc = np.ascontiguousarray(np.asarray(w4C, dtype=np.float32).reshape(D, 1))
    wq = np.ascontiguousarray(np.asarray(w4Q, dtype=np.float32).reshape(D, 1))
    in_maps = []
    for i in range(N_CORES):
        sl = slice(BPC * i, BPC * (i + 1))
        in_maps.append(
            {
                "C": C[sl],
                "Q": Q[sl],
                "negQm": negQm[sl],
                "Cmf": Cmf[sl],
                "wmlu": wmlu,
                "wc": wc,
                "wq": wq,
            }
        )
    return in_maps


def kernel(C, Q, Cmask, Qmask, w4C, w4Q, w4mlu, bias):
    # bias is a scalar added to every logit; it cancels in both softmaxes and
    # never reaches the output, so it is accepted and ignored.
    from concourse.bass_utils import run_bass_kernel_spmd

    nc = _get_program()
    in_maps = _shard_inputs(C, Q, Cmask, Qmask, w4C, w4Q, w4mlu)
    res = run_bass_kernel_spmd(nc, in_maps, list(range(N_CORES)))
    out = np.concatenate([res.results[i]["out"] for i in range(N_CORES)], axis=0)
    return out.astype(np.float32)


# revision 8
# speedup vs baseline: 902.1367x; 153.9847x over previous
"""CQAttention (QANet context-query attention) on 8 Trainium2 NeuronCores.

Full inputs in, full output out. Data-parallel over batch B=32 -> 4 batches
per core. See _build_program() for the per-core Bass/Tile program.

Math notes (vs the jax reference):
  - `bias` and the cross-terms sub0/sub1 that are constant along a softmax
    axis drop out of that softmax; sub1 enters S1's logits as a per-q bias,
    sub0 enters S2's logits via folding w4C into the rhs of the S2 matmul.
  - S1 = softmax_q(sub2 + sub1 + Qmaskbias): computed transposed [q, c];
    row-sum rs[c] over q via an all-ones [q,128] lhsT matmul (result arrives
    pre-broadcast over 128 partitions); 1/rs applied to the final A^T/Bt^T.
  - S2 = softmax_c(sub2 + sub0 + Cmaskbias): computed [c, q]; the c-mask is
    applied multiplicatively (on transposed C rows and on the column-sum
    matmul's lhsT), so exp needs no per-chunk bias.
  - A^T = Qt^T-weighted sums, Bt^T = S1t-weighted V, V = S2^T @ Ct, all via
    PE matmuls with the contraction dim on partitions.
"""

import os
import sys

for _p in ("/opt/trn_rl_repo", "/root/.axon_site/_ro/trn_rl_repo"):
    if os.path.isdir(_p) and _p not in sys.path:
        sys.path.insert(0, _p)

import numpy as np

N_CORES = 8
B_FULL = 32
BPC = B_FULL // N_CORES  # batches per core
D = 128
LC = 2048
LQ = 256
NEG_BIG = -30000.0

_CACHE = {}


def _build_program(repeat=1):
    import concourse.mybir as mybir
    import concourse.tile as tile
    from concourse import bacc
    from concourse.masks import make_identity

    f32 = mybir.dt.float32
    f32r = mybir.dt.float32r
    AF = mybir.ActivationFunctionType
    OP = mybir.AluOpType

    nc = bacc.Bacc("TRN2", target_bir_lowering=False, debug=False)

    Cd = nc.dram_tensor("C", [BPC, D, LC], f32, kind="ExternalInput")
    Qd = nc.dram_tensor("Q", [BPC, D, LQ], f32r, kind="ExternalInput")
    nQd = nc.dram_tensor("negQm", [BPC, D, 2], f32, kind="ExternalInput")
    Cmd = nc.dram_tensor("Cmf", [BPC, D, 16], f32r, kind="ExternalInput")
    wmlud = nc.dram_tensor("wmlu", [D, 1], f32, kind="ExternalInput")
    wcd = nc.dram_tensor("wc", [D, 1], f32, kind="ExternalInput")
    wqd = nc.dram_tensor("wq", [D, 1], f32, kind="ExternalInput")
    outd = nc.dram_tensor("out", [BPC, 4 * D, LC], f32, kind="ExternalOutput")

    def v32(ap):
        return ap.bitcast(f32)

    with tile.TileContext(nc) as tc:
        with (
            tc.tile_pool(name="const", bufs=1) as constp,
            tc.tile_pool(name="big", bufs=2) as sb,
            tc.tile_pool(name="small", bufs=2) as sbs,
            tc.tile_pool(name="psbig", bufs=2, space="PSUM") as psbig,
            tc.tile_pool(name="psct", bufs=2, space="PSUM") as psct,
            tc.tile_pool(name="pssm", bufs=2, space="PSUM") as pssm,
            tc.tile_pool(name="dram", bufs=2, space="DRAM") as dramp,
        ):
            ident = constp.tile([128, 128], f32)
            make_identity(nc, ident[:])
            ones32 = constp.tile([128, 128], f32)
            nc.vector.memset(ones32[:], 1.0)
            onesm = constp.tile([128, 128], f32r)
            nc.vector.tensor_copy(onesm[:], ones32[:])
            wmlu = constp.tile([D, 1], f32)
            nc.sync.dma_start(out=wmlu[:], in_=wmlud.ap())
            wc = constp.tile([D, 1], f32)
            nc.sync.dma_start(out=wc[:], in_=wcd.ap())
            wq = constp.tile([D, 1], f32)
            nc.sync.dma_start(out=wq[:], in_=wqd.ap())

            import contextlib
            loop_cm = tc.For_i(0, repeat) if repeat > 1 else contextlib.nullcontext()
            with loop_cm:
              for b in range(BPC):
                # ---------------- loads ----------------
                Cb = sb.tile([128, LC], f32, tag="Cb")
                nc.sync.dma_start(out=Cb[:], in_=Cd.ap()[b, :, :])
                Qb = sbs.tile([128, LQ], f32r, tag="Qb")
                nc.sync.dma_start(out=Qb[:], in_=Qd.ap()[b, :, :])
                nQm = sbs.tile([128, 2], f32, tag="nQm")
                nc.sync.dma_start(out=nQm[:], in_=nQd.ap()[b, :, :])
                Cmc = sbs.tile([128, 16], f32r, tag="Cmc")
                nc.sync.dma_start(out=Cmc[:], in_=Cmd.ap()[b, :, :])

                # rounded copy of C for f32r matmul consumption
                Cb_r = sb.tile([128, LC], f32r, tag="Cb_r")
                nc.vector.tensor_copy(Cb_r[:], Cb[:])

                # ---------------- small prep ----------------
                QbW = sbs.tile([128, LQ], f32r, tag="QbW")
                nc.vector.tensor_scalar_mul(out=QbW[:], in0=v32(Qb[:]), scalar1=wmlu[:])
                Qw = sbs.tile([128, LQ], f32r, tag="Qw")
                nc.vector.tensor_scalar(
                    out=Qw[:], in0=v32(Qb[:]), scalar1=wmlu[:], scalar2=wc[:],
                    op0=OP.mult, op1=OP.add,
                )

                # sub1[q] = sum_d Q[d,q] * w4Q[d]  -> [q, 1] per q-chunk
                ps_sub1 = pssm.tile([128, 2], f32, tag="sm")
                for qj in range(2):
                    nc.tensor.matmul(
                        ps_sub1[:, qj : qj + 1],
                        lhsT=v32(Qb[:, 128 * qj : 128 * (qj + 1)]),
                        rhs=wq[:],
                        start=True, stop=True,
                    )
                biasQ = sbs.tile([128, 2], f32, tag="biasQ")
                nc.vector.tensor_add(out=biasQ[:], in0=nQm[:], in1=ps_sub1[:])

                # Qt [q, d] (2 chunks side by side)
                ps_qt = pssm.tile([128, 256], f32, tag="sm")
                for qj in range(2):
                    nc.tensor.transpose(
                        ps_qt[:, 128 * qj : 128 * (qj + 1)],
                        in_=v32(Qb[:, 128 * qj : 128 * (qj + 1)]),
                        identity=ident[:],
                    )
                QtS = sbs.tile([128, 256], f32r, tag="QtS")
                nc.vector.tensor_copy(QtS[:], ps_qt[:])

                # ---------------- CT (transposed, c-masked C) ----------------
                CTm = []
                for g in range(4):
                    ps_ct = psct.tile([128, 512], f32, tag="ct")
                    for k in range(4):
                        j = 4 * g + k
                        nc.tensor.transpose(
                            ps_ct[:, 128 * k : 128 * (k + 1)],
                            in_=Cb[:, 128 * j : 128 * (j + 1)],
                            identity=ident[:],
                        )
                    ctm = sb.tile([128, 512], f32r, tag=f"CTm{g}")
                    if g % 2 == 0:
                        nc.scalar.copy(out=ctm[:], in_=ps_ct[:])
                    else:
                        nc.vector.tensor_copy(ctm[:], ps_ct[:])
                    for k in range(4):
                        j = 4 * g + k
                        nc.vector.tensor_scalar_mul(
                            out=ctm[:, 128 * k : 128 * (k + 1)],
                            in0=v32(ctm[:, 128 * k : 128 * (k + 1)]),
                            scalar1=v32(Cmc[:, j : j + 1]),
                        )
                    CTm.append(ctm)

                # ---------------- S1 side: N1t [q, c] ----------------
                N1t = []
                for qj in range(2):
                    n1 = sb.tile([128, LC], f32r, tag=f"N1t{qj}")
                    for h in range(2):
                        ps = psbig.tile([128, 1024], f32, tag="bigmm")
                        for n5 in range(2):
                            c0 = 1024 * h + 512 * n5
                            nc.tensor.matmul(
                                ps[:, 512 * n5 : 512 * (n5 + 1)],
                                lhsT=QbW[:, 128 * qj : 128 * (qj + 1)],
                                rhs=Cb_r[:, c0 : c0 + 512],
                                start=True, stop=True,
                            )
                        nc.scalar.activation(
                            out=n1[:, 1024 * h : 1024 * (h + 1)],
                            in_=ps[:],
                            func=AF.Exp,
                            bias=biasQ[:, qj : qj + 1],
                            scale=1.0,
                        )
                    N1t.append(n1)

                # rs[c] broadcast over partitions, then 1/rs
                RBr = sb.tile([128, LC], f32, tag="RBr")
                for h in range(2):
                    ps = psbig.tile([128, 1024], f32, tag="bigmm")
                    for n5 in range(2):
                        c0 = 1024 * h + 512 * n5
                        for qj in range(2):
                            nc.tensor.matmul(
                                ps[:, 512 * n5 : 512 * (n5 + 1)],
                                lhsT=onesm[:],
                                rhs=N1t[qj][:, c0 : c0 + 512],
                                start=(qj == 0), stop=(qj == 1),
                            )
                    nc.vector.reciprocal_approx_fast(
                        out=RBr[:, 1024 * h : 1024 * (h + 1)], in_=ps[:]
                    )

                # ---------------- S2 side: N2 [c, q] ----------------
                N2 = []
                for s in range(2):
                    n2 = sb.tile([128, 8, 256], f32r, tag=f"N2{s}")
                    for h in range(2):
                        ps = psbig.tile([128, 1024], f32, tag="bigmm")
                        for k in range(4):
                            j = 8 * s + 4 * h + k
                            nc.tensor.matmul(
                                ps[:, 256 * k : 256 * (k + 1)],
                                lhsT=Cb_r[:, 128 * j : 128 * (j + 1)],
                                rhs=Qw[:],
                                start=True, stop=True,
                            )
                        nc.scalar.activation(
                            out=n2[:, 4 * h : 4 * (h + 1), :],
                            in_=ps[:],
                            func=AF.Exp,
                        )
                    N2.append(n2)

                # cs[q] = sum_c Cm[c] * N2[c, q]  -> [1, 256] psum
                ps_cs = pssm.tile([1, 256], f32, tag="sm")
                for j in range(16):
                    s, jj = divmod(j, 8)
                    nc.tensor.matmul(
                        ps_cs[:],
                        lhsT=Cmc[:, j : j + 1],
                        rhs=N2[s][:, jj, :],
                        start=(j == 0), stop=(j == 15),
                    )
                cs_row = sbs.tile([1, 256], f32, tag="cs_row")
                nc.vector.tensor_copy(cs_row[:], ps_cs[:])
                # reshape [1, 256] -> [128, 2] via a DRAM bounce (strided DRAM
                # read; SBUF partition dim cannot be re-strided)
                dcs = dramp.tile([1, 256], f32, tag="dcs")
                nc.sync.dma_start(out=dcs[:], in_=cs_row[:])
                cs_col = sbs.tile([128, 2], f32, tag="cs_col")
                nc.sync.dma_start(
                    out=cs_col[:],
                    in_=dcs[:].rearrange("a (j p) -> p (a j)", p=128),
                )
                rcs = sbs.tile([128, 2], f32, tag="rcs")
                nc.vector.reciprocal(out=rcs[:], in_=cs_col[:])

                # ---------------- V = S2^T @ Ct  [q, d] ----------------
                ps_vt = pssm.tile([128, 256], f32, tag="sm")
                for j in range(16):
                    s, jj = divmod(j, 8)
                    g, k = divmod(j, 4)
                    nc.tensor.matmul(
                        ps_vt[:],
                        lhsT=CTm[g][:, 128 * k : 128 * (k + 1)],
                        rhs=N2[s][:, jj, :],
                        start=(j == 0), stop=(j == 15),
                    )
                VtS = sbs.tile([128, 256], f32, tag="VtS")
                nc.vector.tensor_copy(VtS[:], ps_vt[:])
                ps_v = pssm.tile([128, 256], f32, tag="sm")
                for qj in range(2):
                    nc.tensor.transpose(
                        ps_v[:, 128 * qj : 128 * (qj + 1)],
                        in_=VtS[:, 128 * qj : 128 * (qj + 1)],
                        identity=ident[:],
                    )
                Vs = sbs.tile([128, 256], f32r, tag="Vs")
                for qj in range(2):
                    nc.vector.tensor_scalar_mul(
                        out=Vs[:, 128 * qj : 128 * (qj + 1)],
                        in0=ps_v[:, 128 * qj : 128 * (qj + 1)],
                        scalar1=rcs[:, qj : qj + 1],
                    )

                # ---------------- outputs ----------------
                # out row-block 0: C itself
                nc.sync.dma_start(out=outd.ap()[b, 0:128, :], in_=Cb[:])

                # A^T and Bt^T raw matmuls + normalization + C products
                o2 = sb.tile([128, LC], f32, tag="o2")
                o4a = sb.tile([128, LC], f32, tag="o4a")
                for h in range(2):
                    ps_at = psbig.tile([128, 1024], f32, tag="bigmm")
                    for n5 in range(2):
                        c0 = 1024 * h + 512 * n5
                        for qj in range(2):
                            nc.tensor.matmul(
                                ps_at[:, 512 * n5 : 512 * (n5 + 1)],
                                lhsT=QtS[:, 128 * qj : 128 * (qj + 1)],
                                rhs=N1t[qj][:, c0 : c0 + 512],
                                start=(qj == 0), stop=(qj == 1),
                            )
                    nc.vector.tensor_mul(
                        out=o2[:, 1024 * h : 1024 * (h + 1)],
                        in0=ps_at[:],
                        in1=RBr[:, 1024 * h : 1024 * (h + 1)],
                    )
                for h in range(2):
                    ps_bt = psbig.tile([128, 1024], f32, tag="bigmm")
                    for n5 in range(2):
                        c0 = 1024 * h + 512 * n5
                        for qj in range(2):
                            nc.tensor.matmul(
                                ps_bt[:, 512 * n5 : 512 * (n5 + 1)],
                                lhsT=Vs[:, 128 * qj : 128 * (qj + 1)],
                                rhs=N1t[qj][:, c0 : c0 + 512],
                                start=(qj == 0), stop=(qj == 1),
                            )
                    nc.vector.tensor_mul(
                        out=o4a[:, 1024 * h : 1024 * (h + 1)],
                        in0=ps_bt[:],
                        in1=RBr[:, 1024 * h : 1024 * (h + 1)],
                    )
                nc.sync.dma_start(out=outd.ap()[b, 128:256, :], in_=o2[:])

                o3 = sb.tile([128, LC], f32, tag="o3", bufs=1)
                nc.vector.tensor_mul(out=o3[:], in0=o2[:], in1=Cb[:])
                nc.sync.dma_start(out=outd.ap()[b, 256:384, :], in_=o3[:])

                o4 = sb.tile([128, LC], f32, tag="o4", bufs=1)
                nc.vector.tensor_mul(out=o4[:], in0=o4a[:], in1=Cb[:])
                nc.sync.dma_start(out=outd.ap()[b, 384:512, :], in_=o4[:])

    nc.compile()
    return nc


def _get_program(repeat=1):
    key = f"nc{repeat}"
    if key not in _CACHE:
        _CACHE[key] = _build_program(repeat)
    return _CACHE[key]


def _shard_inputs(C, Q, Cmask, Qmask, w4C, w4Q, w4mlu):
    C = np.ascontiguousarray(C, dtype=np.float32)
    Q = np.ascontiguousarray(Q, dtype=np.float32)
    negQm = (NEG_BIG * (1.0 - Qmask.astype(np.float32))).astype(np.float32)
    # [B, LQ] -> [B, 2, 128] -> [B, 128, 2]
    negQm = np.ascontiguousarray(negQm.reshape(B_FULL, 2, 128).transpose(0, 2, 1))
    Cmf = Cmask.astype(np.float32).reshape(B_FULL, 16, 128).transpose(0, 2, 1)
    Cmf = np.ascontiguousarray(Cmf)
    wmlu = np.ascontiguousarray(np.asarray(w4mlu, dtype=np.float32).reshape(D, 1))
    wc = np.ascontiguousarray(np.asarray(w4C, dtype=np.float32).reshape(D, 1))
    wq = np.ascontiguousarray(np.asarray(w4Q, dtype=np.float32).reshape(D, 1))
    in_maps = []
    for i in range(N_CORES):
        sl = slice(BPC * i, BPC * (i + 1))
        in_maps.append(
            {
                "C": C[sl],
                "Q": Q[sl],
                "negQm": negQm[sl],
                "Cmf": Cmf[sl],
                "wmlu": wmlu,
                "wc": wc,
                "wq": wq,
            }
        )
    return in_maps


def kernel(C, Q, Cmask, Qmask, w4C, w4Q, w4mlu, bias):
    # bias is a scalar added to every logit; it cancels in both softmaxes and
    # never reaches the output, so it is accepted and ignored.
    from concourse.bass_utils import run_bass_kernel_spmd

    nc = _get_program()
    in_maps = _shard_inputs(C, Q, Cmask, Qmask, w4C, w4Q, w4mlu)
    res = run_bass_kernel_spmd(nc, in_maps, list(range(N_CORES)))
    out = np.concatenate([res.results[i]["out"] for i in range(N_CORES)], axis=0)
    return out.astype(np.float32)
